# revision 8
# baseline (speedup 1.0000x reference)
"""Trainium2 Bass kernel for masked attention softmax (ragged sequences).

Reference computation (per batch b):
    qp[k]   = sum_q query[b,0,q] * w[k,q]
    att[s]  = sum_k qp[k] * keys[b,s,k]
    score   = where(s < seq_len[b], att, NEG_INF)
    out[b]  = softmax(score)            # over s axis

Strategy:
  - Data-parallel over batch across 8 cores (512 batches/core, 4 tiles of 128).
  - Ragged trick: sort batches by seq_len descending (host-side), deal
    round-robin to cores so tile slot j has the same max length on every
    core; bake that extent into the kernel and only load/compute
    keys[:, :s_ext_j, :].  Saves ~half of the DMA+compute.
  - Per 128-batch tile (batch on partitions):
      * qp via one PE matmul (query tile pre-transposed on host)
      * att via one DVE tensor_tensor_reduce per s position (fused mul+sum)
      * mask penalty via tensor_scalar (iota >= len) * NEG_INF
      * softmax: reduce_max(negate) -> ACT exp(bias=-max, accum_out=sum)
        -> reciprocal -> tensor_scalar_mul
  - Host scatters per-core outputs back via inverse permutation; rows with
    seq_len == 0 are uniform 1/S (reference softmax of all-equal scores).
"""

import sys

import numpy as np

sys.path.insert(0, "/opt/trn_rl_repo")

import concourse.bass as bass
import concourse.tile as tile
from concourse import bacc, mybir
from concourse.bass_utils import run_bass_kernel_spmd


def _install_trace_shims():
    """The agent image lacks ``antenv.axon_hooks``, so trace=True silently
    degrades.  Recreate the module and register the ctypes NTFF hook from
    trn_agent_boot; also make artifact upload failure non-fatal."""
    try:
        import types

        import antenv
        from concourse import bass_utils as _bu

        if "antenv.axon_hooks" not in sys.modules:
            mod = types.ModuleType("antenv.axon_hooks")
            mod._hook = None
            mod.set_axon_ntff_profile_hook = lambda h: setattr(mod, "_hook", h)
            mod.get_axon_ntff_profile_hook = lambda: mod._hook
            sys.modules["antenv.axon_hooks"] = mod
            antenv.axon_hooks = mod
            from trn_agent_boot.trn_boot import _ntff_profile_via_ctypes

            mod.set_axon_ntff_profile_hook(
                _ntff_profile_via_ctypes("/opt/axon/libaxon_pjrt.so")
            )

        _orig_upload = _bu.upload_artifacts

        def _safe_upload(tmpdir):
            try:
                return _orig_upload(tmpdir)
            except Exception:
                return "local://" + str(tmpdir)

        _bu.upload_artifacts = _safe_upload
    except Exception:
        pass


_install_trace_shims()

B, S, KD, QD = 4096, 200, 128, 128
NCORES = 8
P = 128
PB = B // NCORES           # batches per core
NTILES = PB // P           # partition tiles per core
NEG_INF = float(-(2**32) + 1)
CH = 50                    # s-positions per keys DMA chunk
GP_FRAC = 0.52             # fraction of s-range multiplied on GpSimd

LAST_RESULTS = None
_nc_cache = {}


def _round8(x):
    return ((int(x) + 7) // 8) * 8


def _build(s_exts):
    f32 = mybir.dt.float32
    # Bacc (not raw Bass): its compile() pass splits multi-semaphore waits
    # into EventSemaphore instructions (TRN2 allows <=1 wait per instruction)
    # and moves matmul waits onto ldweights.
    nc = bacc.Bacc("TRN2", target_bir_lowering=False, debug=False)
    keys_d = nc.dram_tensor("keys", [PB, S, KD], f32, kind="ExternalInput")
    # qw[j] = [qT_j | wT] fused so each tile's matmul depends on ONE dma
    # (walrus limits sync-wait commands on Matmult/LDWEIGHTS).
    qw_d = nc.dram_tensor("qw", [NTILES, QD, P + KD], f32, kind="ExternalInput")
    slen_d = nc.dram_tensor("slen", [NTILES, P, 1], f32, kind="ExternalInput")
    out_d = nc.dram_tensor("out", [PB, S], f32, kind="ExternalOutput")
    iota_np = np.tile(np.arange(S, dtype=np.float32), (P, 1))
    iota_d = nc.inline_tensor(iota_np, name="iota")

    with tile.TileContext(nc) as tc:
        with (
            tc.tile_pool(name="const", bufs=1) as constp,
            tc.tile_pool(name="keys", bufs=3) as keysp,
            tc.tile_pool(name="small", bufs=2) as smallp,
            tc.tile_pool(name="scr", bufs=2) as scrp,
            tc.tile_pool(name="gp", bufs=2) as gpp,
            tc.tile_pool(name="psum", bufs=4, space=bass.MemorySpace.PSUM) as psump,
        ):
            iota = constp.tile([P, S], f32)
            nc.sync.dma_start(iota[:], iota_d[:])

            for j in range(NTILES):
                E = s_exts[j]

                qw = smallp.tile([QD, P + KD], f32, tag="qw")
                nc.sync.dma_start(qw[:], qw_d[j])
                slen = smallp.tile([P, 1], f32, tag="slen")
                nc.sync.dma_start(slen[:], slen_d[j])

                # qp[b,k] = sum_q qT[q,b] * wT[q,k]
                qp_ps = psump.tile([P, KD], f32, tag="qp_ps")
                nc.tensor.matmul(
                    qp_ps[:], qw[:, :P], qw[:, P : P + KD], start=True, stop=True
                )
                qp = smallp.tile([P, KD], f32, tag="qp")
                nc.scalar.copy(qp[:], qp_ps[:])

                # penalty[b,s] = (s >= len_b) * NEG_INF
                pen = smallp.tile([P, E], f32, tag="pen")
                nc.vector.tensor_scalar(
                    pen[:],
                    iota[:, :E],
                    slen[:],
                    NEG_INF,
                    op0=mybir.AluOpType.is_ge,
                    op1=mybir.AluOpType.mult,
                )

                # chunk schedule: small first chunk on tile 0 so DVE starts
                # early; last GP_FRAC of each tile's s-range is computed by
                # GpSimd (big tensor_tensor multiply, DVE only reduces).
                chunks = []
                c0 = 16 if j == 0 else 0
                if c0:
                    chunks.append((0, c0))
                while c0 < E:
                    ch = min(CH, E - c0)
                    chunks.append((c0, ch))
                    c0 += ch
                gp_target = int(GP_FRAC * E)
                gp_elems = 0
                kinds = []
                for c0, ch in reversed(chunks):
                    if gp_elems < gp_target and ch >= 16:
                        kinds.append("gp")
                        gp_elems += ch
                    else:
                        kinds.append("dve")
                kinds.reverse()

                att = smallp.tile([P, E], f32, tag="att")
                for (c0, ch), kind in zip(chunks, kinds):
                    kt = keysp.tile([P, CH, KD], f32, tag="kt")
                    nc.sync.dma_start(
                        kt[:, :ch, :], keys_d[j * P : (j + 1) * P, c0 : c0 + ch, :]
                    )
                    if kind == "gp":
                        # GpSimd: prod = keys * qp (qp broadcast along s);
                        # DVE: att[:, c0:c0+ch] = sum_k prod
                        gprod = gpp.tile([P, CH, KD], f32, tag="gprod")
                        qb = qp[:].unsqueeze(1).broadcast_to((P, ch, KD))
                        nc.gpsimd.tensor_tensor(
                            gprod[:, :ch, :], kt[:, :ch, :], qb,
                            op=mybir.AluOpType.mult,
                        )
                        nc.vector.tensor_reduce(
                            att[:, c0 : c0 + ch],
                            gprod[:, :ch, :],
                            axis=mybir.AxisListType.X,
                            op=mybir.AluOpType.add,
                        )
                    else:
                        for s in range(ch):
                            # (keys_s * 1.0) * qp, accum_out = sum_k -> att[:, s]
                            # (scalar_tensor_tensor lowers to the native
                            # TensorScalarPtr opcode; tensor_tensor_reduce's
                            # custom ISA opcode crashes the runtime here.)
                            scr = scrp.tile([P, KD], f32, tag="scr")
                            nc.vector.scalar_tensor_tensor(
                                scr[:],
                                kt[:, s, :],
                                1.0,
                                qp[:],
                                op0=mybir.AluOpType.mult,
                                op1=mybir.AluOpType.mult,
                                accum_out=att[:, c0 + s : c0 + s + 1],
                            )

                score = smallp.tile([P, E], f32, tag="score")
                nc.vector.tensor_add(score[:], att[:], pen[:])
                nmax = smallp.tile([P, 1], f32, tag="nmax")
                nc.vector.reduce_max(
                    nmax[:], score[:], axis=mybir.AxisListType.X, negate=True
                )
                e_t = smallp.tile([P, E], f32, tag="e")
                ssum = smallp.tile([P, 1], f32, tag="ssum")
                nc.scalar.activation(
                    e_t[:],
                    score[:],
                    mybir.ActivationFunctionType.Exp,
                    bias=nmax[:],
                    scale=1.0,
                    accum_out=ssum[:],
                )
                rec = smallp.tile([P, 1], f32, tag="rec")
                nc.vector.reciprocal(rec[:], ssum[:])
                o_t = smallp.tile([P, E], f32, tag="o")
                # final scale on the (otherwise idle) ACT engine
                nc.scalar.mul(o_t[:], e_t[:], rec[:])
                nc.sync.dma_start(out_d[j * P : (j + 1) * P, 0:E], o_t[:])
    nc.compile()
    return nc


def _prep(query, keys, seq_len, w):
    query = np.ascontiguousarray(np.asarray(query), dtype=np.float32)
    keys = np.ascontiguousarray(np.asarray(keys), dtype=np.float32)
    w = np.ascontiguousarray(np.asarray(w), dtype=np.float32)
    lens = np.asarray(seq_len).reshape(B).astype(np.int64)

    order = np.argsort(-lens, kind="stable")
    gp = NCORES * P  # batches per tile slot across all cores
    slot_max = [int(lens[order[j * gp : (j + 1) * gp]].max()) for j in range(NTILES)]
    s_exts = tuple(min(S, max(8, _round8(m))) for m in slot_max)

    perms = []
    for c in range(NCORES):
        perms.append(
            np.concatenate(
                [order[j * gp : (j + 1) * gp][c::NCORES] for j in range(NTILES)]
            )
        )

    wT = np.ascontiguousarray(w.T)
    in_maps = []
    for c in range(NCORES):
        pc = perms[c]
        qT = query[pc, 0, :].reshape(NTILES, P, QD).transpose(0, 2, 1)
        qw = np.empty((NTILES, QD, P + KD), dtype=np.float32)
        qw[:, :, :P] = qT
        qw[:, :, P:] = wT[None]
        in_maps.append(
            {
                "keys": np.ascontiguousarray(keys[pc]),
                "qw": qw,
                "slen": np.ascontiguousarray(
                    lens[pc].astype(np.float32).reshape(NTILES, P, 1)
                ),
            }
        )
    return lens, s_exts, perms, in_maps


def kernel(query, keys, seq_len, w):
    global LAST_RESULTS
    lens, s_exts, perms, in_maps = _prep(query, keys, seq_len, w)

    nc = _nc_cache.get(s_exts)
    if nc is None:
        nc = _build(s_exts)
        _nc_cache[s_exts] = nc

    res = run_bass_kernel_spmd(nc, in_maps, core_ids=list(range(NCORES)))
    LAST_RESULTS = res

    out = np.zeros((B, S), dtype=np.float32)
    for c in range(NCORES):
        dev = np.asarray(res.results[c]["out"])
        pc = perms[c]
        for j in range(NTILES):
            E = s_exts[j]
            rows = pc[j * P : (j + 1) * P]
            out[rows, :E] = dev[j * P : (j + 1) * P, :E]
    out[lens == 0, :] = np.float32(1.0 / S)
    return out


# revision 9
# speedup vs baseline: 1.4647x; 1.4647x over previous
"""Trainium2 Bass kernel for masked attention softmax (ragged sequences).

Reference computation (per batch b):
    qp[k]   = sum_q query[b,0,q] * w[k,q]
    att[s]  = sum_k qp[k] * keys[b,s,k]
    score   = where(s < seq_len[b], att, NEG_INF)
    out[b]  = softmax(score)            # over s axis

Strategy:
  - Data-parallel over batch across 8 cores (512 batches/core, 4 tiles of 128).
  - Ragged trick: sort batches by seq_len descending (host-side), deal
    round-robin to cores so tile slot j has the same max length on every
    core; bake that extent into the kernel and only load/compute
    keys[:, :s_ext_j, :].  Saves ~half of the DMA+compute.
  - Per 128-batch tile (batch on partitions):
      * qp via one PE matmul (query tile pre-transposed on host)
      * att via one DVE tensor_tensor_reduce per s position (fused mul+sum)
      * mask penalty via tensor_scalar (iota >= len) * NEG_INF
      * softmax: reduce_max(negate) -> ACT exp(bias=-max, accum_out=sum)
        -> reciprocal -> tensor_scalar_mul
  - Host scatters per-core outputs back via inverse permutation; rows with
    seq_len == 0 are uniform 1/S (reference softmax of all-equal scores).
"""

import sys

import numpy as np

sys.path.insert(0, "/opt/trn_rl_repo")

import concourse.bass as bass
import concourse.tile as tile
from concourse import bacc, mybir
from concourse.bass_utils import run_bass_kernel_spmd


def _install_trace_shims():
    """The agent image lacks ``antenv.axon_hooks``, so trace=True silently
    degrades.  Recreate the module and register the ctypes NTFF hook from
    trn_agent_boot; also make artifact upload failure non-fatal."""
    try:
        import types

        import antenv
        from concourse import bass_utils as _bu

        if "antenv.axon_hooks" not in sys.modules:
            mod = types.ModuleType("antenv.axon_hooks")
            mod._hook = None
            mod.set_axon_ntff_profile_hook = lambda h: setattr(mod, "_hook", h)
            mod.get_axon_ntff_profile_hook = lambda: mod._hook
            sys.modules["antenv.axon_hooks"] = mod
            antenv.axon_hooks = mod
            from trn_agent_boot.trn_boot import _ntff_profile_via_ctypes

            mod.set_axon_ntff_profile_hook(
                _ntff_profile_via_ctypes("/opt/axon/libaxon_pjrt.so")
            )

        _orig_upload = _bu.upload_artifacts

        def _safe_upload(tmpdir):
            try:
                return _orig_upload(tmpdir)
            except Exception:
                return "local://" + str(tmpdir)

        _bu.upload_artifacts = _safe_upload
    except Exception:
        pass


_install_trace_shims()

B, S, KD, QD = 4096, 200, 128, 128
NCORES = 8
P = 128
PB = B // NCORES           # batches per core
NTILES = PB // P           # partition tiles per core
NEG_INF = float(-(2**32) + 1)
CH = 50                    # s-positions per keys DMA chunk
GP_FRAC = 0.0              # GpSimd offload disabled: its SBUF traffic slows concurrent DVE ops ~4x

LAST_RESULTS = None
_nc_cache = {}


def _round8(x):
    return ((int(x) + 7) // 8) * 8


def _build(s_exts):
    f32 = mybir.dt.float32
    # Bacc (not raw Bass): its compile() pass splits multi-semaphore waits
    # into EventSemaphore instructions (TRN2 allows <=1 wait per instruction)
    # and moves matmul waits onto ldweights.
    nc = bacc.Bacc("TRN2", target_bir_lowering=False, debug=False)
    keys_d = nc.dram_tensor("keys", [PB, S, KD], f32, kind="ExternalInput")
    # qw[j] = [qT_j | wT] fused so each tile's matmul depends on ONE dma
    # (walrus limits sync-wait commands on Matmult/LDWEIGHTS).
    qw_d = nc.dram_tensor("qw", [NTILES, QD, P + KD], f32, kind="ExternalInput")
    slen_d = nc.dram_tensor("slen", [NTILES, P, 1], f32, kind="ExternalInput")
    out_d = nc.dram_tensor("out", [PB, S], f32, kind="ExternalOutput")
    iota_np = np.tile(np.arange(S, dtype=np.float32), (P, 1))
    iota_d = nc.inline_tensor(iota_np, name="iota")

    with tile.TileContext(nc) as tc:
        with (
            tc.tile_pool(name="const", bufs=1) as constp,
            tc.tile_pool(name="keys", bufs=3) as keysp,
            tc.tile_pool(name="small", bufs=2) as smallp,
            tc.tile_pool(name="scr", bufs=2) as scrp,
            tc.tile_pool(name="gp", bufs=2) as gpp,
            tc.tile_pool(name="psum", bufs=4, space=bass.MemorySpace.PSUM) as psump,
        ):
            iota = constp.tile([P, S], f32)
            nc.sync.dma_start(iota[:], iota_d[:])

            for j in range(NTILES):
                E = s_exts[j]

                qw = smallp.tile([QD, P + KD], f32, tag="qw")
                nc.sync.dma_start(qw[:], qw_d[j])
                slen = smallp.tile([P, 1], f32, tag="slen")
                nc.sync.dma_start(slen[:], slen_d[j])

                # qp[b,k] = sum_q qT[q,b] * wT[q,k]
                qp_ps = psump.tile([P, KD], f32, tag="qp_ps")
                nc.tensor.matmul(
                    qp_ps[:], qw[:, :P], qw[:, P : P + KD], start=True, stop=True
                )
                qp = smallp.tile([P, KD], f32, tag="qp")
                nc.scalar.copy(qp[:], qp_ps[:])

                # penalty[b,s] = (s >= len_b) * NEG_INF
                pen = smallp.tile([P, E], f32, tag="pen")
                nc.vector.tensor_scalar(
                    pen[:],
                    iota[:, :E],
                    slen[:],
                    NEG_INF,
                    op0=mybir.AluOpType.is_ge,
                    op1=mybir.AluOpType.mult,
                )

                # chunk schedule: small first chunk on tile 0 so DVE starts
                # early; last GP_FRAC of each tile's s-range is computed by
                # GpSimd (big tensor_tensor multiply, DVE only reduces).
                chunks = []
                c0 = 16 if j == 0 else 0
                if c0:
                    chunks.append((0, c0))
                while c0 < E:
                    ch = min(CH, E - c0)
                    chunks.append((c0, ch))
                    c0 += ch
                gp_target = int(GP_FRAC * E)
                gp_elems = 0
                kinds = []
                for c0, ch in reversed(chunks):
                    if gp_elems < gp_target and ch >= 16:
                        kinds.append("gp")
                        gp_elems += ch
                    else:
                        kinds.append("dve")
                kinds.reverse()

                att = smallp.tile([P, E], f32, tag="att")
                for (c0, ch), kind in zip(chunks, kinds):
                    kt = keysp.tile([P, CH, KD], f32, tag="kt")
                    nc.sync.dma_start(
                        kt[:, :ch, :], keys_d[j * P : (j + 1) * P, c0 : c0 + ch, :]
                    )
                    if kind == "gp":
                        # GpSimd: prod = keys * qp (qp broadcast along s);
                        # DVE: att[:, c0:c0+ch] = sum_k prod
                        gprod = gpp.tile([P, CH, KD], f32, tag="gprod")
                        qb = qp[:].unsqueeze(1).broadcast_to((P, ch, KD))
                        nc.gpsimd.tensor_tensor(
                            gprod[:, :ch, :], kt[:, :ch, :], qb,
                            op=mybir.AluOpType.mult,
                        )
                        nc.vector.tensor_reduce(
                            att[:, c0 : c0 + ch],
                            gprod[:, :ch, :],
                            axis=mybir.AxisListType.X,
                            op=mybir.AluOpType.add,
                        )
                    else:
                        for s in range(ch):
                            # (keys_s * 1.0) * qp, accum_out = sum_k -> att[:, s]
                            # (scalar_tensor_tensor lowers to the native
                            # TensorScalarPtr opcode; tensor_tensor_reduce's
                            # custom ISA opcode crashes the runtime here.)
                            scr = scrp.tile([P, KD], f32, tag="scr")
                            nc.vector.scalar_tensor_tensor(
                                scr[:],
                                kt[:, s, :],
                                1.0,
                                qp[:],
                                op0=mybir.AluOpType.mult,
                                op1=mybir.AluOpType.mult,
                                accum_out=att[:, c0 + s : c0 + s + 1],
                            )

                score = smallp.tile([P, E], f32, tag="score")
                nc.vector.tensor_add(score[:], att[:], pen[:])
                nmax = smallp.tile([P, 1], f32, tag="nmax")
                nc.vector.reduce_max(
                    nmax[:], score[:], axis=mybir.AxisListType.X, negate=True
                )
                e_t = smallp.tile([P, E], f32, tag="e")
                ssum = smallp.tile([P, 1], f32, tag="ssum")
                nc.scalar.activation(
                    e_t[:],
                    score[:],
                    mybir.ActivationFunctionType.Exp,
                    bias=nmax[:],
                    scale=1.0,
                    accum_out=ssum[:],
                )
                rec = smallp.tile([P, 1], f32, tag="rec")
                nc.vector.reciprocal(rec[:], ssum[:])
                o_t = smallp.tile([P, E], f32, tag="o")
                # final scale on the (otherwise idle) ACT engine
                nc.scalar.mul(o_t[:], e_t[:], rec[:])
                nc.sync.dma_start(out_d[j * P : (j + 1) * P, 0:E], o_t[:])
    nc.compile()
    return nc


def _prep(query, keys, seq_len, w):
    query = np.ascontiguousarray(np.asarray(query), dtype=np.float32)
    keys = np.ascontiguousarray(np.asarray(keys), dtype=np.float32)
    w = np.ascontiguousarray(np.asarray(w), dtype=np.float32)
    lens = np.asarray(seq_len).reshape(B).astype(np.int64)

    order = np.argsort(-lens, kind="stable")
    gp = NCORES * P  # batches per tile slot across all cores
    slot_max = [int(lens[order[j * gp : (j + 1) * gp]].max()) for j in range(NTILES)]
    s_exts = tuple(min(S, max(8, _round8(m))) for m in slot_max)

    perms = []
    for c in range(NCORES):
        perms.append(
            np.concatenate(
                [order[j * gp : (j + 1) * gp][c::NCORES] for j in range(NTILES)]
            )
        )

    wT = np.ascontiguousarray(w.T)
    in_maps = []
    for c in range(NCORES):
        pc = perms[c]
        qT = query[pc, 0, :].reshape(NTILES, P, QD).transpose(0, 2, 1)
        qw = np.empty((NTILES, QD, P + KD), dtype=np.float32)
        qw[:, :, :P] = qT
        qw[:, :, P:] = wT[None]
        in_maps.append(
            {
                "keys": np.ascontiguousarray(keys[pc]),
                "qw": qw,
                "slen": np.ascontiguousarray(
                    lens[pc].astype(np.float32).reshape(NTILES, P, 1)
                ),
            }
        )
    return lens, s_exts, perms, in_maps


def kernel(query, keys, seq_len, w):
    global LAST_RESULTS
    lens, s_exts, perms, in_maps = _prep(query, keys, seq_len, w)

    nc = _nc_cache.get(s_exts)
    if nc is None:
        nc = _build(s_exts)
        _nc_cache[s_exts] = nc

    res = run_bass_kernel_spmd(nc, in_maps, core_ids=list(range(NCORES)))
    LAST_RESULTS = res

    out = np.zeros((B, S), dtype=np.float32)
    for c in range(NCORES):
        dev = np.asarray(res.results[c]["out"])
        pc = perms[c]
        for j in range(NTILES):
            E = s_exts[j]
            rows = pc[j * P : (j + 1) * P]
            out[rows, :E] = dev[j * P : (j + 1) * P, :E]
    out[lens == 0, :] = np.float32(1.0 / S)
    return out
